# revision 1
# baseline (speedup 1.0000x reference)
"""Trainium2 Bass kernel for nn_Aggregate (segment_reduce).

Reference computation:
    cat_idx = idx_inputs[:, argmax(softmax(cat_mask))]          # [N]
    agg     = segment_sum(inputs[:, 16:], cat_idx, 100000)       # [S, 128]
    out     = agg[cat_idx][:, top32(softmax(numeric_mask))] * conf

Strategy (no collectives, no gpsimd):
  * Only the 32 top-k numeric columns survive to the output, and segment_sum
    is linear per column -> select those 32 columns FIRST (4x less data),
    and fold the conf scaling into them.
  * Sort rows by segment on the host.  After sorting, each segment's rows
    are one contiguous run; the segment total is the forward *segmented
    cumulative sum* (tensor_tensor_scan: state = m*state + x, m=0 at run
    starts, fp32 state) evaluated at the run's last row.
  * The device streams in (x, m) in bf16, runs the scan in-place (the
    cumsum overwrites the mask region of the same SBUF tile), and streams
    the raw cumsum straight back out.  The HOST picks the run-end values
    out of the returned cumsum - it already does a full-size fancy-index
    gather to un-sort the output, so this adds no asymptotic host work.
  * Shard: the sorted rows are cut at segment boundaries into
    8 cores x 4 partition-groups x 16 windows = 512 independent chunks of
    ~1953 rows, each padded to K=2048.  Every chunk is a fully independent
    scan, and all 16 windows are SBUF-resident (no buffer reuse), so the
    only device-side dependencies are load_w -> scan_w -> store_w.

Everything data-dependent (masks, run-end positions, slot bookkeeping) is
precomputed on the host; the device graph is static.
"""

import sys
import types

import ml_dtypes
import numpy as np

if "/opt/trn_rl_repo" not in sys.path:
    sys.path.insert(0, "/opt/trn_rl_repo")

import concourse.bacc as bacc
import concourse.mybir as mybir
import concourse.tile as tile

# ----------------------------------------------------------------------------
# problem constants (hardcoded per spec)
N_ROWS = 1_000_000
NUM_CAT = 16
NUM_NUMERICS = 128
N_ARY = 32
NUM_SEGMENTS = 100_000

NCORES = 8
SEQ_PER_CORE = 4                     # partition-groups per core
NW = 16                              # windows (independent chunks) per group
NCHUNK = NCORES * SEQ_PER_CORE * NW  # 512 global chunks
CHUNK_NOMINAL = N_ROWS // NCHUNK     # 1953
P = 128
K = 2048                             # padded chunk length = window size
F = K * NW                           # 32768 free columns per core
BX = K * 2                           # X (bf16) bytes per window per partition
BM = K * 2                           # mask/cumsum (bf16) bytes per window
BW = BX + BM                         # fused window bytes

BF16 = ml_dtypes.bfloat16

_dt = mybir.dt

_CACHE: dict = {}


def _ensure_axon_hooks():
    """bass_utils imports antenv.axon_hooks for trace=True; provide a shim
    so the import never fails (hook stays None unless a profiler sets it)."""
    if "antenv.axon_hooks" in sys.modules:
        return sys.modules["antenv.axon_hooks"]
    mod = types.ModuleType("antenv.axon_hooks")
    hook = [None]
    mod.set_axon_ntff_profile_hook = lambda h: hook.__setitem__(0, h)
    mod.get_axon_ntff_profile_hook = lambda: hook[0]
    sys.modules["antenv.axon_hooks"] = mod
    return mod


def build_bass():
    """Build + compile the (SPMD, per-core identical) Bass graph once."""
    if "nc" in _CACHE:
        return _CACHE["nc"]
    nc = bacc.Bacc("TRN2", target_bir_lowering=False, debug=False,
                   num_devices=NCORES)
    xm_ext = nc.dram_tensor("xm", [P, NW * BW], _dt.uint8,
                            kind="ExternalInput").ap()
    out_ext = nc.dram_tensor("out", [P, NW * BM], _dt.uint8,
                             kind="ExternalOutput").ap()

    with tile.TileContext(nc) as tc:
        with tc.tile_pool(name="xp", bufs=1) as xp:
            for w in range(NW):
                ft = xp.tile([P, BW], _dt.uint8, tag=f"w{w}")
                nc.sync.dma_start(out=ft[:], in_=xm_ext[:, w * BW:(w + 1) * BW])
                xt = ft[:, 0:BX].bitcast(_dt.bfloat16)
                mc = ft[:, BX:BW].bitcast(_dt.bfloat16)
                # in-place: the cumsum lands where the mask was (1:1 bytes,
                # each element read before it is overwritten)
                nc.vector.tensor_tensor_scan(
                    out=mc, data0=mc, data1=xt, initial=0.0,
                    op0=mybir.AluOpType.mult, op1=mybir.AluOpType.add,
                )
                nc.scalar.dma_start(out=out_ext[:, w * BM:(w + 1) * BM],
                                    in_=ft[:, BX:BW])
    nc.compile()
    _CACHE["nc"] = nc
    return nc


def _softmax64(v):
    v = np.asarray(v, dtype=np.float64)
    e = np.exp(v - v.max())
    return e / e.sum()


def prepare(inputs, idx_inputs, cat_mask, numeric_mask):
    """Host-side prep: top-k, column select + conf scale, sort, shard.

    Returns (in_maps, meta): in_maps[i] feeds core i; meta carries the
    indices postprocess needs to pull run-end cumsums out of the returned
    tables and expand them to rows.
    """
    cat_mask = np.asarray(cat_mask)
    numeric_mask = np.asarray(numeric_mask)
    cm = _softmax64(cat_mask)
    ti = int(np.argmax(cm))                     # top_k(1) -> first max
    top_cat_val = cm[ti]
    nm = _softmax64(numeric_mask)
    order = np.argsort(-nm, kind="stable")[:N_ARY]   # descending, ties->low idx
    conf = ((nm[order] + top_cat_val) / 2.0).astype(np.float32)

    seg = np.ascontiguousarray(np.asarray(idx_inputs)[:, ti]).astype(np.int32)
    perm = np.argsort(seg, kind="stable")
    seg_s = seg[perm]

    inputs = np.asarray(inputs)
    sel = inputs[:, NUM_CAT + order].astype(np.float32) * conf[None, :]
    xs = sel[perm].astype(BF16)                  # [N, 32] sorted, bf16

    # run bookkeeping
    isstart = np.empty(N_ROWS, dtype=bool)
    isstart[0] = True
    isstart[1:] = seg_s[1:] != seg_s[:-1]
    rank_s = np.cumsum(isstart) - 1              # [N] slot of each row's run
    start_pos = np.flatnonzero(isstart)          # [R] run start row
    nruns = len(start_pos)
    end_pos = np.empty(nruns, dtype=np.int64)    # [R] run end row (inclusive)
    end_pos[:-1] = start_pos[1:] - 1
    end_pos[-1] = N_ROWS - 1

    # original-order slot of every row (for the final host gather)
    r_orig = np.empty(N_ROWS, dtype=np.int64)
    r_orig[perm] = rank_s

    # chunk cuts at run starts
    cut = np.empty(NCHUNK + 1, dtype=np.int64)
    cut[0], cut[NCHUNK] = 0, N_ROWS
    for j in range(1, NCHUNK):
        n = j * CHUNK_NOMINAL
        cut[j] = np.searchsorted(seg_s, seg_s[n], side="left")
    lens = np.diff(cut)
    assert lens.max() <= K, f"chunk too long: {lens.max()} > {K}"

    # first run of each chunk
    run_cut = np.searchsorted(start_pos, cut[:-1])
    run_cut = np.append(run_cut, nruns)

    # mask: 1 where same segment as previous row
    m_all = np.ones(N_ROWS, dtype=BF16)
    m_all[isstart] = 0

    in_maps = []
    core_gidx = []      # per core: (g_arr, off_arr) into the returned [4,32,F]
    for i in range(NCORES):
        X = np.zeros((P, F), dtype=BF16)
        M = np.ones((P, F), dtype=BF16)          # pad: m=1 keeps state flat
        gs, offs = [], []
        for g in range(SEQ_PER_CORE):
            for w in range(NW):
                j = (i * SEQ_PER_CORE + g) * NW + w
                c0, c1 = int(cut[j]), int(cut[j + 1])
                ln = c1 - c0
                X[g * 32:(g + 1) * 32, w * K:w * K + ln] = xs[c0:c1].T
                M[g * 32:(g + 1) * 32, w * K:w * K + ln] = m_all[c0:c1][None, :]
                ends = end_pos[run_cut[j]:run_cut[j + 1]] - c0   # chunk-local
                gs.append(np.full(len(ends), g, dtype=np.int64))
                offs.append(w * K + ends)
        core_gidx.append((np.concatenate(gs), np.concatenate(offs)))
        XM = np.concatenate(
            [X.view(np.uint8).reshape(P, NW, BX),
             M.view(np.uint8).reshape(P, NW, BM)],
            axis=2).reshape(P, NW * BW)
        in_maps.append({"xm": np.ascontiguousarray(XM)})
    meta = {"r_orig": r_orig, "core_gidx": core_gidx, "nruns": nruns}
    return in_maps, meta


def postprocess(results, meta):
    """Pull run-end cumsums from the returned tables, expand to rows."""
    table = np.empty((meta["nruns"], N_ARY), dtype=np.float32)
    pos = 0
    for i in range(NCORES):
        C = results[i]["out"].view(BF16).astype(np.float32)      # [P, F]
        C = C.reshape(SEQ_PER_CORE, 32, F)
        g_arr, off_arr = meta["core_gidx"][i]
        vals = C[g_arr, :, off_arr]                              # [R_i, 32]
        table[pos:pos + len(g_arr)] = vals
        pos += len(g_arr)
    assert pos == meta["nruns"]
    return table[meta["r_orig"]]


def run(in_maps, trace=False, trace_kwargs=None):
    _ensure_axon_hooks()
    from concourse.bass_utils import run_bass_kernel_spmd
    nc = build_bass()
    return run_bass_kernel_spmd(nc, in_maps, core_ids=list(range(NCORES)),
                                trace=trace, **(trace_kwargs or {}))


def kernel(inputs, idx_inputs, cat_mask, numeric_mask):
    in_maps, meta = prepare(inputs, idx_inputs, cat_mask, numeric_mask)
    res = run(in_maps, trace=False)
    return postprocess(res.results, meta)



# revision 2
# speedup vs baseline: 1.1099x; 1.1099x over previous
"""Trainium2 Bass kernel for nn_Aggregate (segment_reduce).

Reference computation:
    cat_idx = idx_inputs[:, argmax(softmax(cat_mask))]          # [N]
    agg     = segment_sum(inputs[:, 16:], cat_idx, 100000)       # [S, 128]
    out     = agg[cat_idx][:, top32(softmax(numeric_mask))] * conf

Strategy (v2 — unsegmented coarse scan + lattice extraction):
  * Only the 32 top-k numeric columns survive to the output, and segment_sum
    is linear per column -> select those 32 columns FIRST (4x less data)
    and fold the conf scaling into them.
  * Sort rows by segment on the host.  Each segment is then one contiguous
    run; its sum is S[end] - S[start-1] where S is a plain (unsegmented)
    per-stream prefix sum -- no masks on the device at all.  The scan state
    and the boundary differences are fp32 on-device, so there is no
    catastrophic cancellation; only the final output is rounded (fp16).
  * 4x coarsening with ZERO vector-engine cost: each run is padded to a
    multiple of 4 rows and split into 4 "phase" planes (rows 4c+0..4c+3).
    The four planes are DMA'd onto the same SBUF tile with the SDMA
    engines' inline CCE adder (gpsimd dma_start accum_op=add), so the
    element the DVE scans is already the sum of 4 consecutive rows.  The
    scan (the stock tensor_tensor_scan runs at ~2 cycles/column) therefore
    touches 4x fewer columns.
  * Output compaction: runs are bucketed by coarse length l=ceil(len/4)
    and dealt uniformly to 8 cores x 4 partition-groups (dummy runs pad
    each bucket to a multiple of 32), so within a bucket the run-end
    prefix values form a regular lattice of stride l.  One strided
    tensor_sub per bucket computes all its segment sums (S[end]-S[end-l])
    straight into a compact fp16 output tile: ~0.8 MB leaves each core
    instead of the full 8 MB cumsum.
  * Host does only routing (sort, bucket, deal, gather) - every add that
    touches row data happens on the device.

Everything data-dependent (bucket geometry, lattice offsets) is baked into
the compiled graph; build_bass() therefore runs after prepare().
"""

import sys
import types

import ml_dtypes
import numpy as np

if "/opt/trn_rl_repo" not in sys.path:
    sys.path.insert(0, "/opt/trn_rl_repo")

import concourse.bacc as bacc
import concourse.mybir as mybir
import concourse.tile as tile

# ----------------------------------------------------------------------------
# problem constants (hardcoded per spec)
N_ROWS = 1_000_000
NUM_CAT = 16
NUM_NUMERICS = 128
N_ARY = 32
NUM_SEGMENTS = 100_000

NCORES = 8
GROUPS = 4                    # partition-groups per core (32 feats each)
NSTREAM = NCORES * GROUPS     # 32 independent scan streams
PH = 4                        # coarsening factor == phase planes
NWIN = 8                      # pipeline windows per core

BF16 = ml_dtypes.bfloat16
F16 = np.float16

_dt = mybir.dt

_CACHE: dict = {}


def _ensure_axon_hooks():
    """bass_utils imports antenv.axon_hooks for trace=True; provide a shim
    so the import never fails (hook stays None unless a profiler sets it)."""
    if "antenv.axon_hooks" in sys.modules:
        return sys.modules["antenv.axon_hooks"]
    mod = types.ModuleType("antenv.axon_hooks")
    hook = [None]
    mod.set_axon_ntff_profile_hook = lambda h: hook.__setitem__(0, h)
    mod.get_axon_ntff_profile_hook = lambda: hook[0]
    sys.modules["antenv.axon_hooks"] = mod
    return mod


def _softmax64(v):
    v = np.asarray(v, dtype=np.float64)
    e = np.exp(v - v.max())
    return e / e.sum()


def prepare(inputs, idx_inputs, cat_mask, numeric_mask):
    """Host-side prep: top-k, column select + conf scale, sort, bucket by
    coarse run length, deal runs to 32 streams, build phase planes.

    Returns (in_maps, meta); also stashes the device-graph geometry in
    _CACHE["geo"] for build_bass().
    """
    cat_mask = np.asarray(cat_mask)
    numeric_mask = np.asarray(numeric_mask)
    cm = _softmax64(cat_mask)
    ti = int(np.argmax(cm))                     # top_k(1) -> first max
    top_cat_val = cm[ti]
    nm = _softmax64(numeric_mask)
    order = np.argsort(-nm, kind="stable")[:N_ARY]   # descending, ties->low idx
    conf = ((nm[order] + top_cat_val) / 2.0).astype(np.float32)

    seg = np.ascontiguousarray(np.asarray(idx_inputs)[:, ti]).astype(np.int32)
    perm = np.argsort(seg, kind="stable")
    seg_s = seg[perm]

    inputs = np.asarray(inputs)
    sel = inputs[:, NUM_CAT + order].astype(np.float32) * conf[None, :]
    xs = sel[perm].astype(BF16)                  # [N, 32] sorted rows, bf16

    # ---- run bookkeeping ----------------------------------------------
    isstart = np.empty(N_ROWS, dtype=bool)
    isstart[0] = True
    isstart[1:] = seg_s[1:] != seg_s[:-1]
    rank_s = np.cumsum(isstart) - 1              # [N] run index of each row
    start_pos = np.flatnonzero(isstart)          # [R]
    nruns = len(start_pos)
    lens = np.empty(nruns, dtype=np.int64)
    lens[:-1] = np.diff(start_pos)
    lens[-1] = N_ROWS - start_pos[-1]
    seg_of_run = seg_s[start_pos]                # [R]
    lp = (lens + PH - 1) // PH                   # coarse slot length per run

    # ---- bucket by coarse length, deal to 32 streams ------------------
    # stream s <-> (core = s // GROUPS, group = s % GROUPS)
    blens = np.unique(lp)
    s_of_run = np.empty(nruns, dtype=np.int64)
    k_of_run = np.empty(nruns, dtype=np.int64)   # slot index within bucket
    bkt_of_run = np.empty(nruns, dtype=np.int64)
    buckets = []                                 # (l, q, B, O) per bucket
    base = 1                                     # coarse col 0 = zero column
    out_base = 0
    for bi, l in enumerate(blens):
        ridx = np.flatnonzero(lp == l)
        m = len(ridx)
        q = -(-m // NSTREAM)                     # slots per stream
        # slot grid [q, NSTREAM]; run j -> (k = j // NSTREAM, s = j % NSTREAM)
        s_of_run[ridx] = np.arange(m) % NSTREAM
        k_of_run[ridx] = np.arange(m) // NSTREAM
        bkt_of_run[ridx] = bi
        buckets.append((int(l), int(q), int(base), int(out_base)))
        base += q * l
        out_base += q
    C4 = base
    Q = out_base
    Wc = -(-C4 // NWIN)
    Wc = (Wc + 7) // 8 * 8                       # round window to mult of 8
    C4pad = Wc * NWIN

    bucket_B = np.array([b[2] for b in buckets], dtype=np.int64)
    bucket_O = np.array([b[3] for b in buckets], dtype=np.int64)
    bucket_L = np.array([b[0] for b in buckets], dtype=np.int64)
    off_of_run = bucket_B[bkt_of_run] + k_of_run * bucket_L[bkt_of_run]
    outcol_of_run = bucket_O[bkt_of_run] + k_of_run

    # ---- scatter sorted rows into per-stream phase-resolved planes ----
    big = np.zeros((NSTREAM, C4pad * PH, N_ARY), dtype=BF16)
    within = np.arange(N_ROWS, dtype=np.int64) - start_pos[rank_s]
    srow = s_of_run[rank_s]
    posrow = PH * off_of_run[rank_s] + within
    big.reshape(-1, N_ARY)[srow * (C4pad * PH) + posrow] = xs

    # [NSTREAM, C4pad, PH, 32] -> [cores, PH, groups, 32feat, C4pad]
    planes = big.reshape(NCORES, GROUPS, C4pad, PH, N_ARY)
    planes = planes.transpose(0, 3, 1, 4, 2)     # [8, PH, 4, 32, C4pad]
    planes = np.ascontiguousarray(planes).reshape(NCORES, PH, 128, C4pad)

    in_maps = []
    for i in range(NCORES):
        in_maps.append({"xin": planes[i].view(np.uint8)})

    _CACHE["geo"] = {"C4pad": C4pad, "Wc": Wc, "Q": Q, "buckets": buckets}
    meta = {
        "seg": seg,
        "seg_of_run": seg_of_run,
        "core_of_run": s_of_run // GROUPS,
        "group_of_run": s_of_run % GROUPS,
        "outcol_of_run": outcol_of_run,
        "Q": Q,
    }
    return in_maps, meta


def build_bass():
    """Build + compile the (SPMD, per-core identical) Bass graph.

    Geometry (window size, extraction lattice) comes from prepare()'s
    stash, so prepare() must run first.
    """
    if "nc" in _CACHE:
        return _CACHE["nc"]
    geo = _CACHE["geo"]
    C4pad, Wc, Q, buckets = geo["C4pad"], geo["Wc"], geo["Q"], geo["buckets"]

    nc = bacc.Bacc("TRN2", target_bir_lowering=False, debug=False,
                   num_devices=NCORES)
    xin = nc.dram_tensor("xin", [PH, 128, C4pad * 2], _dt.uint8,
                         kind="ExternalInput").ap()
    xout = nc.dram_tensor("out", [128, Q * 2], _dt.uint8,
                          kind="ExternalOutput").ap()

    with tile.TileContext(nc) as tc:
        with tc.tile_pool(name="pp", bufs=1) as pool, \
             tc.tile_pool(name="xp", bufs=4) as xpool:
            S = pool.tile([128, C4pad], _dt.float32, tag="S")
            z = pool.tile([128, Wc], _dt.bfloat16, tag="z")
            ot = pool.tile([128, Q], _dt.float16, tag="o")
            nc.vector.memset(z[:], 0.0)
            for w in range(NWIN):
                a, b = w * Wc, (w + 1) * Wc
                xt = xpool.tile([128, Wc], _dt.bfloat16, tag="x")
                nc.sync.dma_start(out=xt[:],
                                  in_=xin[0][:, a * 2:b * 2].bitcast(_dt.bfloat16))
                for p in range(1, PH):
                    nc.gpsimd.dma_start(
                        out=xt[:],
                        in_=xin[p][:, a * 2:b * 2].bitcast(_dt.bfloat16),
                        accum_op=mybir.AluOpType.add)
                init = 0.0 if w == 0 else S[:, a - 1:a]
                nc.vector.tensor_tensor_scan(
                    out=S[:, a:b], data0=z[:], data1=xt[:], initial=init,
                    op0=mybir.AluOpType.add, op1=mybir.AluOpType.add)
            for (l, q, B, O) in buckets:
                e0 = B + l - 1
                nc.vector.tensor_sub(
                    out=ot[:, O:O + q],
                    in0=S[:, e0:e0 + q * l:l],
                    in1=S[:, B - 1:B - 1 + q * l:l])
            nc.scalar.dma_start(out=xout[:], in_=ot[:].bitcast(_dt.uint8))
    nc.compile()
    _CACHE["nc"] = nc
    return nc


def postprocess(results, meta):
    """Pull per-run sums from the compacted device outputs, expand to rows."""
    table = np.zeros((NUM_SEGMENTS, N_ARY), dtype=np.float32)
    core = meta["core_of_run"]
    group = meta["group_of_run"]
    outcol = meta["outcol_of_run"]
    for i in range(NCORES):
        O = results[i]["out"].view(F16).astype(np.float32)       # [128, Q]
        O = O.reshape(GROUPS, 32, meta["Q"])
        m = core == i
        table[meta["seg_of_run"][m]] = O[group[m], :, outcol[m]]
    return table[meta["seg"]]


def run(in_maps, trace=False, trace_kwargs=None):
    _ensure_axon_hooks()
    from concourse.bass_utils import run_bass_kernel_spmd
    nc = build_bass()
    return run_bass_kernel_spmd(nc, in_maps, core_ids=list(range(NCORES)),
                                trace=trace, **(trace_kwargs or {}))


def kernel(inputs, idx_inputs, cat_mask, numeric_mask):
    in_maps, meta = prepare(inputs, idx_inputs, cat_mask, numeric_mask)
    res = run(in_maps, trace=False)
    return postprocess(res.results, meta)


# revision 5
# speedup vs baseline: 1.2765x; 1.1501x over previous
"""Trainium2 Bass kernel for nn_Aggregate (segment_reduce).

Reference computation:
    cat_idx = idx_inputs[:, argmax(softmax(cat_mask))]          # [N]
    agg     = segment_sum(inputs[:, 16:], cat_idx, 100000)       # [S, 128]
    out     = agg[cat_idx][:, top32(softmax(numeric_mask))] * conf

Strategy (v2 — unsegmented coarse scan + lattice extraction):
  * Only the 32 top-k numeric columns survive to the output, and segment_sum
    is linear per column -> select those 32 columns FIRST (4x less data)
    and fold the conf scaling into them.
  * Sort rows by segment on the host.  Each segment is then one contiguous
    run; its sum is S[end] - S[start-1] where S is a plain (unsegmented)
    per-stream prefix sum -- no masks on the device at all.  The scan state
    and the boundary differences are fp32 on-device, so there is no
    catastrophic cancellation; only the final output is rounded (fp16).
  * 4x coarsening: each run is padded to a multiple of 4 rows and split
    into 4 "phase" planes (rows 4c+0..4c+3).  The stock tensor_tensor_scan
    runs at ~2 cycles/column, so scanning at 1/4 resolution is the win.
    The 4->2 phase reduction is split across engines: one bf16 2x-mode
    tensor_add on the DVE (p0+=p1), one on the otherwise-idle GPSIMD
    (p2+=p3), and the final 2->1 add is FREE: the scan's recurrence is
    state = (data0 + state) + data1, so it consumes both partial streams
    directly.
  * Output compaction: runs are bucketed by coarse length l=ceil(len/4)
    and dealt uniformly to 8 cores x 4 partition-groups (dummy runs pad
    each bucket to a multiple of 32), so within a bucket the run-end
    prefix values form a regular lattice of stride l.  One strided
    tensor_sub per bucket computes all its segment sums (S[end]-S[end-l])
    straight into a compact fp16 output tile: ~0.8 MB leaves each core
    instead of the full 8 MB cumsum.
  * Host does only routing (sort, bucket, deal, gather) - every add that
    touches row data happens on the device.

Everything data-dependent (bucket geometry, lattice offsets) is baked into
the compiled graph; build_bass() therefore runs after prepare().
"""

import sys
import types

import ml_dtypes
import numpy as np

if "/opt/trn_rl_repo" not in sys.path:
    sys.path.insert(0, "/opt/trn_rl_repo")

import concourse.bacc as bacc
import concourse.mybir as mybir
import concourse.tile as tile

# ----------------------------------------------------------------------------
# problem constants (hardcoded per spec)
N_ROWS = 1_000_000
NUM_CAT = 16
NUM_NUMERICS = 128
N_ARY = 32
NUM_SEGMENTS = 100_000

NCORES = 8
GROUPS = 4                    # partition-groups per core (32 feats each)
NSTREAM = NCORES * GROUPS     # 32 independent scan streams
PH = 4                        # coarsening factor == phase planes
NWIN = 8                      # pipeline windows per core

BF16 = ml_dtypes.bfloat16
F16 = np.float16

_dt = mybir.dt

_CACHE: dict = {}


def _ensure_axon_hooks():
    """bass_utils imports antenv.axon_hooks for trace=True; provide a shim
    so the import never fails (hook stays None unless a profiler sets it)."""
    if "antenv.axon_hooks" in sys.modules:
        return sys.modules["antenv.axon_hooks"]
    mod = types.ModuleType("antenv.axon_hooks")
    hook = [None]
    mod.set_axon_ntff_profile_hook = lambda h: hook.__setitem__(0, h)
    mod.get_axon_ntff_profile_hook = lambda: hook[0]
    sys.modules["antenv.axon_hooks"] = mod
    return mod


def _softmax64(v):
    v = np.asarray(v, dtype=np.float64)
    e = np.exp(v - v.max())
    return e / e.sum()


def prepare(inputs, idx_inputs, cat_mask, numeric_mask):
    """Host-side prep: top-k, column select + conf scale, sort, bucket by
    coarse run length, deal runs to 32 streams, build phase planes.

    Returns (in_maps, meta); also stashes the device-graph geometry in
    _CACHE["geo"] for build_bass().
    """
    cat_mask = np.asarray(cat_mask)
    numeric_mask = np.asarray(numeric_mask)
    cm = _softmax64(cat_mask)
    ti = int(np.argmax(cm))                     # top_k(1) -> first max
    top_cat_val = cm[ti]
    nm = _softmax64(numeric_mask)
    order = np.argsort(-nm, kind="stable")[:N_ARY]   # descending, ties->low idx
    conf = ((nm[order] + top_cat_val) / 2.0).astype(np.float32)

    seg = np.ascontiguousarray(np.asarray(idx_inputs)[:, ti]).astype(np.int32)
    perm = np.argsort(seg, kind="stable")
    seg_s = seg[perm]

    inputs = np.asarray(inputs)
    sel = inputs[:, NUM_CAT + order].astype(np.float32) * conf[None, :]
    xs = sel[perm].astype(BF16)                  # [N, 32] sorted rows, bf16

    # ---- run bookkeeping ----------------------------------------------
    isstart = np.empty(N_ROWS, dtype=bool)
    isstart[0] = True
    isstart[1:] = seg_s[1:] != seg_s[:-1]
    rank_s = np.cumsum(isstart) - 1              # [N] run index of each row
    start_pos = np.flatnonzero(isstart)          # [R]
    nruns = len(start_pos)
    lens = np.empty(nruns, dtype=np.int64)
    lens[:-1] = np.diff(start_pos)
    lens[-1] = N_ROWS - start_pos[-1]
    seg_of_run = seg_s[start_pos]                # [R]
    lp = (lens + PH - 1) // PH                   # coarse slot length per run

    # ---- bucket by coarse length, deal to 32 streams ------------------
    # stream s <-> (core = s // GROUPS, group = s % GROUPS)
    blens = np.unique(lp)
    s_of_run = np.empty(nruns, dtype=np.int64)
    k_of_run = np.empty(nruns, dtype=np.int64)   # slot index within bucket
    bkt_of_run = np.empty(nruns, dtype=np.int64)
    buckets = []                                 # (l, q, B, O) per bucket
    base = 1                                     # coarse col 0 = zero column
    out_base = 0
    for bi, l in enumerate(blens):
        ridx = np.flatnonzero(lp == l)
        m = len(ridx)
        q = -(-m // NSTREAM)                     # slots per stream
        # slot grid [q, NSTREAM]; run j -> (k = j // NSTREAM, s = j % NSTREAM)
        s_of_run[ridx] = np.arange(m) % NSTREAM
        k_of_run[ridx] = np.arange(m) // NSTREAM
        bkt_of_run[ridx] = bi
        buckets.append((int(l), int(q), int(base), int(out_base)))
        base += q * l
        out_base += q
    C4 = base
    Q = out_base
    Wc = -(-C4 // NWIN)
    Wc = (Wc + 7) // 8 * 8                       # round window to mult of 8
    C4pad = Wc * NWIN

    bucket_B = np.array([b[2] for b in buckets], dtype=np.int64)
    bucket_O = np.array([b[3] for b in buckets], dtype=np.int64)
    bucket_L = np.array([b[0] for b in buckets], dtype=np.int64)
    off_of_run = bucket_B[bkt_of_run] + k_of_run * bucket_L[bkt_of_run]
    outcol_of_run = bucket_O[bkt_of_run] + k_of_run

    # ---- scatter sorted rows into per-stream phase-resolved planes ----
    big = np.zeros((NSTREAM, C4pad * PH, N_ARY), dtype=BF16)
    within = np.arange(N_ROWS, dtype=np.int64) - start_pos[rank_s]
    srow = s_of_run[rank_s]
    posrow = PH * off_of_run[rank_s] + within
    big.reshape(-1, N_ARY)[srow * (C4pad * PH) + posrow] = xs

    # [NSTREAM, C4pad, PH, 32] -> [cores, PH, groups, 32feat, C4pad]
    planes = big.reshape(NCORES, GROUPS, C4pad, PH, N_ARY)
    planes = planes.transpose(0, 3, 1, 4, 2)     # [8, PH, 4, 32, C4pad]
    planes = np.ascontiguousarray(planes).reshape(NCORES, PH, 128, C4pad)

    in_maps = []
    for i in range(NCORES):
        in_maps.append({"xin": planes[i].view(np.uint8)})

    _CACHE["geo"] = {"C4pad": C4pad, "Wc": Wc, "Q": Q, "buckets": buckets}
    meta = {
        "seg": seg,
        "seg_of_run": seg_of_run,
        "core_of_run": s_of_run // GROUPS,
        "group_of_run": s_of_run % GROUPS,
        "outcol_of_run": outcol_of_run,
        "Q": Q,
    }
    return in_maps, meta


def build_bass():
    """Build + compile the (SPMD, per-core identical) Bass graph.

    Geometry (window size, extraction lattice) comes from prepare()'s
    stash, so prepare() must run first.
    """
    if "nc" in _CACHE:
        return _CACHE["nc"]
    geo = _CACHE["geo"]
    C4pad, Wc, Q, buckets = geo["C4pad"], geo["Wc"], geo["Q"], geo["buckets"]

    nc = bacc.Bacc("TRN2", target_bir_lowering=False, debug=False,
                   num_devices=NCORES)
    xin = nc.dram_tensor("xin", [PH, 128, C4pad * 2], _dt.uint8,
                         kind="ExternalInput").ap()
    xout = nc.dram_tensor("out", [128, Q * 2], _dt.uint8,
                          kind="ExternalOutput").ap()

    with tile.TileContext(nc) as tc:
        with tc.tile_pool(name="pp", bufs=1) as pool, \
             tc.tile_pool(name="xp", bufs=3) as xpool:
            S = pool.tile([128, C4pad], _dt.float32, tag="S")
            ot = pool.tile([128, Q], _dt.float16, tag="o")
            for w in range(NWIN):
                a, b = w * Wc, (w + 1) * Wc
                ph = [xpool.tile([128, Wc], _dt.bfloat16, tag=f"x{p}",
                                 name=f"ph{p}")
                      for p in range(PH)]
                for p in range(PH):
                    eng = nc.sync if p < 2 else nc.scalar
                    eng.dma_start(out=ph[p][:],
                                  in_=xin[p][:, a * 2:b * 2].bitcast(_dt.bfloat16))
                nc.vector.tensor_add(out=ph[0][:], in0=ph[0][:], in1=ph[1][:])
                nc.gpsimd.tensor_add(out=ph[2][:], in0=ph[2][:], in1=ph[3][:])
                init = 0.0 if w == 0 else S[:, a - 1:a]
                nc.vector.tensor_tensor_scan(
                    out=S[:, a:b], data0=ph[2][:], data1=ph[0][:], initial=init,
                    op0=mybir.AluOpType.add, op1=mybir.AluOpType.add)
            for (l, q, B, O) in buckets:
                e0 = B + l - 1
                nc.vector.tensor_sub(
                    out=ot[:, O:O + q],
                    in0=S[:, e0:e0 + q * l:l],
                    in1=S[:, B - 1:B - 1 + q * l:l])
            nc.scalar.dma_start(out=xout[:], in_=ot[:].bitcast(_dt.uint8))
    nc.compile()
    _CACHE["nc"] = nc
    return nc


def postprocess(results, meta):
    """Pull per-run sums from the compacted device outputs, expand to rows."""
    table = np.zeros((NUM_SEGMENTS, N_ARY), dtype=np.float32)
    core = meta["core_of_run"]
    group = meta["group_of_run"]
    outcol = meta["outcol_of_run"]
    for i in range(NCORES):
        O = results[i]["out"].view(F16).astype(np.float32)       # [128, Q]
        O = O.reshape(GROUPS, 32, meta["Q"])
        m = core == i
        table[meta["seg_of_run"][m]] = O[group[m], :, outcol[m]]
    return table[meta["seg"]]


def run(in_maps, trace=False, trace_kwargs=None):
    _ensure_axon_hooks()
    from concourse.bass_utils import run_bass_kernel_spmd
    nc = build_bass()
    return run_bass_kernel_spmd(nc, in_maps, core_ids=list(range(NCORES)),
                                trace=trace, **(trace_kwargs or {}))


def kernel(inputs, idx_inputs, cat_mask, numeric_mask):
    in_maps, meta = prepare(inputs, idx_inputs, cat_mask, numeric_mask)
    res = run(in_maps, trace=False)
    return postprocess(res.results, meta)


# revision 7
# speedup vs baseline: 1.5056x; 1.1795x over previous
"""Trainium2 Bass kernel for nn_Aggregate (segment_reduce).

Reference computation:
    cat_idx = idx_inputs[:, argmax(softmax(cat_mask))]          # [N]
    agg     = segment_sum(inputs[:, 16:], cat_idx, 100000)       # [S, 128]
    out     = agg[cat_idx][:, top32(softmax(numeric_mask))] * conf

Strategy (v2 — unsegmented coarse scan + lattice extraction):
  * Only the 32 top-k numeric columns survive to the output, and segment_sum
    is linear per column -> select those 32 columns FIRST (4x less data)
    and fold the conf scaling into them.
  * Sort rows by segment on the host.  Each segment is then one contiguous
    run; its sum is S[end] - S[start-1] where S is a plain (unsegmented)
    per-stream prefix sum -- no masks on the device at all.  The scan state
    and the boundary differences are fp32 on-device, so there is no
    catastrophic cancellation; only the final output is rounded (fp16).
  * 4x coarsening: each run is padded to a multiple of 4 rows and split
    into 4 "phase" planes (rows 4c+0..4c+3).  The stock tensor_tensor_scan
    runs at ~2 cycles/column, so scanning at 1/4 resolution is the win.
    The 4->2 phase reduction is split across engines: one bf16 2x-mode
    tensor_add on the DVE (p0+=p1), one on the otherwise-idle GPSIMD
    (p2+=p3), and the final 2->1 add is FREE: the scan's recurrence is
    state = (data0 + state) + data1, so it consumes both partial streams
    directly.
  * Output compaction: runs are bucketed by coarse length l=ceil(len/4)
    and dealt uniformly to 8 cores x 4 partition-groups (dummy runs pad
    each bucket to a multiple of 32), so within a bucket the run-end
    prefix values form a regular lattice of stride l.  One strided
    tensor_sub per bucket computes all its segment sums (S[end]-S[end-l])
    straight into a compact fp16 output tile: ~0.8 MB leaves each core
    instead of the full 8 MB cumsum.
  * Host does only routing (sort, bucket, deal, gather) - every add that
    touches row data happens on the device.

Everything data-dependent (bucket geometry, lattice offsets) is baked into
the compiled graph; build_bass() therefore runs after prepare().
"""

import sys
import types

import ml_dtypes
import numpy as np

if "/opt/trn_rl_repo" not in sys.path:
    sys.path.insert(0, "/opt/trn_rl_repo")

import concourse.bacc as bacc
import concourse.mybir as mybir
import concourse.tile as tile

# ----------------------------------------------------------------------------
# problem constants (hardcoded per spec)
N_ROWS = 1_000_000
NUM_CAT = 16
NUM_NUMERICS = 128
N_ARY = 32
NUM_SEGMENTS = 100_000

NCORES = 8
GROUPS = 4                    # partition-groups per core (32 feats each)
NSTREAM = NCORES * GROUPS     # 32 independent scan streams
PH = 4                        # coarsening factor == phase planes
NWIN = 4                      # pipeline windows per core

BF16 = ml_dtypes.bfloat16
F16 = np.float16

_dt = mybir.dt

_CACHE: dict = {}


def _ensure_axon_hooks():
    """bass_utils imports antenv.axon_hooks for trace=True; provide a shim
    so the import never fails (hook stays None unless a profiler sets it)."""
    if "antenv.axon_hooks" in sys.modules:
        return sys.modules["antenv.axon_hooks"]
    mod = types.ModuleType("antenv.axon_hooks")
    hook = [None]
    mod.set_axon_ntff_profile_hook = lambda h: hook.__setitem__(0, h)
    mod.get_axon_ntff_profile_hook = lambda: hook[0]
    sys.modules["antenv.axon_hooks"] = mod
    return mod


def _softmax64(v):
    v = np.asarray(v, dtype=np.float64)
    e = np.exp(v - v.max())
    return e / e.sum()


def prepare(inputs, idx_inputs, cat_mask, numeric_mask):
    """Host-side prep: top-k, column select + conf scale, sort, bucket by
    coarse run length, deal runs to 32 streams, build phase planes.

    Returns (in_maps, meta); also stashes the device-graph geometry in
    _CACHE["geo"] for build_bass().
    """
    cat_mask = np.asarray(cat_mask)
    numeric_mask = np.asarray(numeric_mask)
    cm = _softmax64(cat_mask)
    ti = int(np.argmax(cm))                     # top_k(1) -> first max
    top_cat_val = cm[ti]
    nm = _softmax64(numeric_mask)
    order = np.argsort(-nm, kind="stable")[:N_ARY]   # descending, ties->low idx
    conf = ((nm[order] + top_cat_val) / 2.0).astype(np.float32)

    seg = np.ascontiguousarray(np.asarray(idx_inputs)[:, ti]).astype(np.int32)
    perm = np.argsort(seg, kind="stable")
    seg_s = seg[perm]

    inputs = np.asarray(inputs)
    sel = inputs[:, NUM_CAT + order].astype(np.float32) * conf[None, :]
    xs = sel[perm].astype(BF16)                  # [N, 32] sorted rows, bf16

    # ---- run bookkeeping ----------------------------------------------
    isstart = np.empty(N_ROWS, dtype=bool)
    isstart[0] = True
    isstart[1:] = seg_s[1:] != seg_s[:-1]
    rank_s = np.cumsum(isstart) - 1              # [N] run index of each row
    start_pos = np.flatnonzero(isstart)          # [R]
    nruns = len(start_pos)
    lens = np.empty(nruns, dtype=np.int64)
    lens[:-1] = np.diff(start_pos)
    lens[-1] = N_ROWS - start_pos[-1]
    seg_of_run = seg_s[start_pos]                # [R]
    lp = (lens + PH - 1) // PH                   # coarse slot length per run

    # ---- bucket by coarse length, deal to 32 streams ------------------
    # stream s <-> (core = s // GROUPS, group = s % GROUPS)
    blens = np.unique(lp)
    s_of_run = np.empty(nruns, dtype=np.int64)
    k_of_run = np.empty(nruns, dtype=np.int64)   # slot index within bucket
    bkt_of_run = np.empty(nruns, dtype=np.int64)
    buckets = []                                 # (l, q, B, O) per bucket
    base = 1                                     # coarse col 0 = zero column
    out_base = 0
    for bi, l in enumerate(blens):
        ridx = np.flatnonzero(lp == l)
        m = len(ridx)
        q = -(-m // NSTREAM)                     # slots per stream
        # slot grid [q, NSTREAM]; run j -> (k = j // NSTREAM, s = j % NSTREAM)
        s_of_run[ridx] = np.arange(m) % NSTREAM
        k_of_run[ridx] = np.arange(m) // NSTREAM
        bkt_of_run[ridx] = bi
        buckets.append((int(l), int(q), int(base), int(out_base)))
        base += q * l
        out_base += q
    C4 = base
    Q = out_base
    Wc = -(-C4 // NWIN)
    Wc = (Wc + 7) // 8 * 8                       # round window to mult of 8
    C4pad = Wc * NWIN

    bucket_B = np.array([b[2] for b in buckets], dtype=np.int64)
    bucket_O = np.array([b[3] for b in buckets], dtype=np.int64)
    bucket_L = np.array([b[0] for b in buckets], dtype=np.int64)
    off_of_run = bucket_B[bkt_of_run] + k_of_run * bucket_L[bkt_of_run]
    outcol_of_run = bucket_O[bkt_of_run] + k_of_run

    # ---- scatter sorted rows into per-stream phase-resolved planes ----
    big = np.zeros((NSTREAM, C4pad * PH, N_ARY), dtype=BF16)
    within = np.arange(N_ROWS, dtype=np.int64) - start_pos[rank_s]
    srow = s_of_run[rank_s]
    posrow = PH * off_of_run[rank_s] + within
    big.reshape(-1, N_ARY)[srow * (C4pad * PH) + posrow] = xs

    # [NSTREAM, C4pad, PH, 32] -> [cores, PH, groups, 32feat, C4pad]
    planes = big.reshape(NCORES, GROUPS, C4pad, PH, N_ARY)
    planes = planes.transpose(0, 3, 1, 4, 2)     # [8, PH, 4, 32, C4pad]
    planes = np.ascontiguousarray(planes).reshape(NCORES, PH, 128, C4pad)

    in_maps = []
    for i in range(NCORES):
        in_maps.append({"xin": planes[i].view(np.uint8)})

    _CACHE["geo"] = {"C4pad": C4pad, "Wc": Wc, "Q": Q, "buckets": buckets}
    meta = {
        "seg": seg,
        "seg_of_run": seg_of_run,
        "core_of_run": s_of_run // GROUPS,
        "group_of_run": s_of_run % GROUPS,
        "outcol_of_run": outcol_of_run,
        "Q": Q,
    }
    return in_maps, meta


def build_bass():
    """Build + compile the (SPMD, per-core identical) Bass graph.

    Geometry (window size, extraction lattice) comes from prepare()'s
    stash, so prepare() must run first.
    """
    if "nc" in _CACHE:
        return _CACHE["nc"]
    geo = _CACHE["geo"]
    C4pad, Wc, Q, buckets = geo["C4pad"], geo["Wc"], geo["Q"], geo["buckets"]

    nc = bacc.Bacc("TRN2", target_bir_lowering=False, debug=False,
                   num_devices=NCORES)
    xin = nc.dram_tensor("xin", [PH, 128, C4pad * 2], _dt.uint8,
                         kind="ExternalInput").ap()
    xout = nc.dram_tensor("out", [128, Q * 2], _dt.uint8,
                          kind="ExternalOutput").ap()

    # last window whose scan a bucket's extraction lattice depends on
    def last_win(B, q, l):
        return min(NWIN - 1, (B + q * l - 1) // Wc)

    subs_after = {w: [] for w in range(NWIN)}
    for bkt in buckets:
        l, q, B, O = bkt
        subs_after[last_win(B, q, l)].append(bkt)

    with tile.TileContext(nc) as tc:
        with tc.tile_pool(name="pp", bufs=1) as pool, \
             tc.tile_pool(name="xp", bufs=3) as xpool, \
             tc.tile_pool(name="ap", bufs=2) as apool:
            S = pool.tile([128, C4pad], _dt.float32, tag="S")
            ot = pool.tile([128, Q], _dt.float16, tag="o")
            for w in range(NWIN):
                a, b = w * Wc, (w + 1) * Wc
                ph = [xpool.tile([128, Wc], _dt.bfloat16, tag=f"x{p}",
                                 name=f"ph{p}")
                      for p in range(PH)]
                for p in range(PH):
                    eng = nc.sync if p < 2 else nc.scalar
                    eng.dma_start(out=ph[p][:],
                                  in_=xin[p][:, a * 2:b * 2].bitcast(_dt.bfloat16))
                a01 = apool.tile([128, Wc], _dt.bfloat16, tag="a01")
                a23 = apool.tile([128, Wc], _dt.bfloat16, tag="a23")
                nc.vector.tensor_add(out=a01[:], in0=ph[0][:], in1=ph[1][:])
                nc.vector.tensor_add(out=a23[:], in0=ph[2][:], in1=ph[3][:])
                init = 0.0 if w == 0 else S[:, a - 1:a]
                nc.vector.tensor_tensor_scan(
                    out=S[:, a:b], data0=a23[:], data1=a01[:], initial=init,
                    op0=mybir.AluOpType.add, op1=mybir.AluOpType.add)
                # strided lattice subtracts run on the otherwise-idle GPSIMD
                # as soon as the last covering scan is done
                for (l, q, B, O) in subs_after[w]:
                    e0 = B + l - 1
                    nc.gpsimd.tensor_sub(
                        out=ot[:, O:O + q],
                        in0=S[:, e0:e0 + q * l:l],
                        in1=S[:, B - 1:B - 1 + q * l:l])
            nc.scalar.dma_start(out=xout[:], in_=ot[:].bitcast(_dt.uint8))
    nc.compile()
    _CACHE["nc"] = nc
    return nc


def postprocess(results, meta):
    """Pull per-run sums from the compacted device outputs, expand to rows."""
    table = np.zeros((NUM_SEGMENTS, N_ARY), dtype=np.float32)
    core = meta["core_of_run"]
    group = meta["group_of_run"]
    outcol = meta["outcol_of_run"]
    for i in range(NCORES):
        O = results[i]["out"].view(F16).astype(np.float32)       # [128, Q]
        O = O.reshape(GROUPS, 32, meta["Q"])
        m = core == i
        table[meta["seg_of_run"][m]] = O[group[m], :, outcol[m]]
    return table[meta["seg"]]


def run(in_maps, trace=False, trace_kwargs=None):
    _ensure_axon_hooks()
    from concourse.bass_utils import run_bass_kernel_spmd
    nc = build_bass()
    return run_bass_kernel_spmd(nc, in_maps, core_ids=list(range(NCORES)),
                                trace=trace, **(trace_kwargs or {}))


def kernel(inputs, idx_inputs, cat_mask, numeric_mask):
    in_maps, meta = prepare(inputs, idx_inputs, cat_mask, numeric_mask)
    res = run(in_maps, trace=False)
    return postprocess(res.results, meta)


# revision 11
# speedup vs baseline: 1.7973x; 1.1937x over previous
"""Trainium2 Bass kernel for nn_Aggregate (segment_reduce).

Reference computation:
    cat_idx = idx_inputs[:, argmax(softmax(cat_mask))]          # [N]
    agg     = segment_sum(inputs[:, 16:], cat_idx, 100000)       # [S, 128]
    out     = agg[cat_idx][:, top32(softmax(numeric_mask))] * conf

Strategy (v2 — unsegmented coarse scan + lattice extraction):
  * Only the 32 top-k numeric columns survive to the output, and segment_sum
    is linear per column -> select those 32 columns FIRST (4x less data)
    and fold the conf scaling into them.
  * Sort rows by segment on the host.  Each segment is then one contiguous
    run; its sum is S[end] - S[start-1] where S is a plain (unsegmented)
    per-stream prefix sum -- no masks on the device at all.  The scan state
    and the boundary differences are fp32 on-device, so there is no
    catastrophic cancellation; only the final output is rounded (fp16).
  * 4x coarsening: each run is padded to a multiple of 4 rows and split
    into 4 "phase" planes (rows 4c+0..4c+3).  The stock tensor_tensor_scan
    runs at ~2 cycles/column, so scanning at 1/4 resolution is the win.
    The 4->2 phase reduction is split across engines: one bf16 2x-mode
    tensor_add on the DVE (p0+=p1), one on the otherwise-idle GPSIMD
    (p2+=p3), and the final 2->1 add is FREE: the scan's recurrence is
    state = (data0 + state) + data1, so it consumes both partial streams
    directly.
  * Output compaction: runs are bucketed by coarse length l=ceil(len/4)
    and dealt uniformly to 8 cores x 4 partition-groups (dummy runs pad
    each bucket to a multiple of 32), so within a bucket the run-end
    prefix values form a regular lattice of stride l.  One strided
    tensor_sub per bucket computes all its segment sums (S[end]-S[end-l])
    straight into a compact fp16 output tile: ~0.8 MB leaves each core
    instead of the full 8 MB cumsum.
  * Host does only routing (sort, bucket, deal, gather) - every add that
    touches row data happens on the device.

Everything data-dependent (bucket geometry, lattice offsets) is baked into
the compiled graph; build_bass() therefore runs after prepare().
"""

import sys
import types

import ml_dtypes
import numpy as np

if "/opt/trn_rl_repo" not in sys.path:
    sys.path.insert(0, "/opt/trn_rl_repo")

import concourse.bacc as bacc
import concourse.mybir as mybir
import concourse.tile as tile
import concourse.dve_ops as dve_ops
from concourse.dve_spec import C0, Spec, Src0, Src1
from concourse.dve_spec import AluOp as DveAluOp
from concourse.dve_spec import scan as dve_scan


def _register_pair_add_scan():
    """Custom DVE op: out[k] = s0 + sum_{j<=k} (in0[j] + in1[j]).

    The stock tensor_tensor_scan routes its state feedback backward
    through the 8-stage pipe and runs at ~2 cycles/element; this Spec's
    scan() combine reads CURR_ALU_OUT (a one-cycle recurrence, no
    bubble) and additionally fuses the final pair-add of the two phase
    streams.  The per-NEFF uop table ships via the standard
    ant.dve_table HLO frontend-attribute path."""
    name = "PAIR_ADD_SCAN_AGG"
    for op in dve_ops.OPS:
        if op.name == name:
            return op
    spec = Spec(
        body=dve_scan(DveAluOp.ADD, Src0 + Src1, init=C0),
        reference=lambda in0, in1, s0, s1, imm2: (
            np.cumsum(in0.astype(np.float32) + in1.astype(np.float32),
                      axis=-1) + np.asarray(s0, dtype=np.float32)),
    )
    op = dve_ops.DveOp(
        name, spec, subdim=False,
        uops_sha={"v3": "8b49596cd428b415", "v4": "9f3b8a1ce4265eb2"},
    )
    dve_ops.OPS.append(op)
    dve_ops.CUSTOM_DVE_SPECS[name] = spec
    dve_ops._SUB_OPCODE_FOR_NAME[name] = (
        max(dve_ops._SUB_OPCODE_FOR_NAME.values()) + 1)
    return op


_PAIR_ADD_SCAN = _register_pair_add_scan()

# ----------------------------------------------------------------------------
# problem constants (hardcoded per spec)
N_ROWS = 1_000_000
NUM_CAT = 16
NUM_NUMERICS = 128
N_ARY = 32
NUM_SEGMENTS = 100_000

NCORES = 8
GROUPS = 4                    # partition-groups per core (32 feats each)
NSTREAM = NCORES * GROUPS     # 32 independent scan streams
PH = 4                        # coarsening factor == phase planes
NWIN = 4                      # pipeline windows per core

BF16 = ml_dtypes.bfloat16
F16 = np.float16

_dt = mybir.dt

_CACHE: dict = {}


def _ensure_axon_hooks():
    """bass_utils imports antenv.axon_hooks for trace=True; provide a shim
    so the import never fails (hook stays None unless a profiler sets it)."""
    if "antenv.axon_hooks" in sys.modules:
        return sys.modules["antenv.axon_hooks"]
    mod = types.ModuleType("antenv.axon_hooks")
    hook = [None]
    mod.set_axon_ntff_profile_hook = lambda h: hook.__setitem__(0, h)
    mod.get_axon_ntff_profile_hook = lambda: hook[0]
    sys.modules["antenv.axon_hooks"] = mod
    return mod


def _softmax64(v):
    v = np.asarray(v, dtype=np.float64)
    e = np.exp(v - v.max())
    return e / e.sum()


def prepare(inputs, idx_inputs, cat_mask, numeric_mask):
    """Host-side prep: top-k, column select + conf scale, sort, bucket by
    coarse run length, deal runs to 32 streams, build phase planes.

    Returns (in_maps, meta); also stashes the device-graph geometry in
    _CACHE["geo"] for build_bass().
    """
    cat_mask = np.asarray(cat_mask)
    numeric_mask = np.asarray(numeric_mask)
    cm = _softmax64(cat_mask)
    ti = int(np.argmax(cm))                     # top_k(1) -> first max
    top_cat_val = cm[ti]
    nm = _softmax64(numeric_mask)
    order = np.argsort(-nm, kind="stable")[:N_ARY]   # descending, ties->low idx
    conf = ((nm[order] + top_cat_val) / 2.0).astype(np.float32)

    seg = np.ascontiguousarray(np.asarray(idx_inputs)[:, ti]).astype(np.int32)
    perm = np.argsort(seg, kind="stable")
    seg_s = seg[perm]

    inputs = np.asarray(inputs)
    sel = inputs[:, NUM_CAT + order].astype(np.float32) * conf[None, :]
    xs = sel[perm].astype(BF16)                  # [N, 32] sorted rows, bf16

    # ---- run bookkeeping ----------------------------------------------
    isstart = np.empty(N_ROWS, dtype=bool)
    isstart[0] = True
    isstart[1:] = seg_s[1:] != seg_s[:-1]
    rank_s = np.cumsum(isstart) - 1              # [N] run index of each row
    start_pos = np.flatnonzero(isstart)          # [R]
    nruns = len(start_pos)
    lens = np.empty(nruns, dtype=np.int64)
    lens[:-1] = np.diff(start_pos)
    lens[-1] = N_ROWS - start_pos[-1]
    seg_of_run = seg_s[start_pos]                # [R]
    lp = (lens + PH - 1) // PH                   # coarse slot length per run

    # ---- bucket by coarse length, deal to 32 streams ------------------
    # stream s <-> (core = s // GROUPS, group = s % GROUPS)
    blens = np.unique(lp)
    s_of_run = np.empty(nruns, dtype=np.int64)
    k_of_run = np.empty(nruns, dtype=np.int64)   # slot index within bucket
    bkt_of_run = np.empty(nruns, dtype=np.int64)
    buckets = []                                 # (l, q, B, O) per bucket
    base = 1                                     # coarse col 0 = zero column
    out_base = 0
    for bi, l in enumerate(blens):
        ridx = np.flatnonzero(lp == l)
        m = len(ridx)
        q = -(-m // NSTREAM)                     # slots per stream
        # slot grid [q, NSTREAM]; run j -> (k = j // NSTREAM, s = j % NSTREAM)
        s_of_run[ridx] = np.arange(m) % NSTREAM
        k_of_run[ridx] = np.arange(m) // NSTREAM
        bkt_of_run[ridx] = bi
        buckets.append((int(l), int(q), int(base), int(out_base)))
        base += q * l
        out_base += q
    C4 = base
    Q = out_base
    Wc = -(-C4 // NWIN)
    Wc = (Wc + 7) // 8 * 8                       # round window to mult of 8
    C4pad = Wc * NWIN

    bucket_B = np.array([b[2] for b in buckets], dtype=np.int64)
    bucket_O = np.array([b[3] for b in buckets], dtype=np.int64)
    bucket_L = np.array([b[0] for b in buckets], dtype=np.int64)
    off_of_run = bucket_B[bkt_of_run] + k_of_run * bucket_L[bkt_of_run]
    outcol_of_run = bucket_O[bkt_of_run] + k_of_run

    # ---- scatter sorted rows into per-stream phase-resolved planes ----
    big = np.zeros((NSTREAM, C4pad * PH, N_ARY), dtype=BF16)
    within = np.arange(N_ROWS, dtype=np.int64) - start_pos[rank_s]
    srow = s_of_run[rank_s]
    posrow = PH * off_of_run[rank_s] + within
    big.reshape(-1, N_ARY)[srow * (C4pad * PH) + posrow] = xs

    # [NSTREAM, C4pad, PH, 32] -> [cores, 128parts, NWIN, PH, Wc]
    planes = big.reshape(NCORES, GROUPS, NWIN, Wc, PH, N_ARY)
    planes = planes.transpose(0, 1, 5, 2, 4, 3)  # [8, g, f, w, p, c]
    planes = np.ascontiguousarray(planes).reshape(NCORES, 128, NWIN, PH, Wc)

    in_maps = []
    for i in range(NCORES):
        in_maps.append({"xin": planes[i].view(np.uint8)})

    _CACHE["geo"] = {"C4pad": C4pad, "Wc": Wc, "Q": Q, "buckets": buckets}
    meta = {
        "seg": seg,
        "seg_of_run": seg_of_run,
        "core_of_run": s_of_run // GROUPS,
        "group_of_run": s_of_run % GROUPS,
        "outcol_of_run": outcol_of_run,
        "Q": Q,
    }
    return in_maps, meta


def build_bass():
    """Build + compile the (SPMD, per-core identical) Bass graph.

    Geometry (window size, extraction lattice) comes from prepare()'s
    stash, so prepare() must run first.
    """
    if "nc" in _CACHE:
        return _CACHE["nc"]
    geo = _CACHE["geo"]
    C4pad, Wc, Q, buckets = geo["C4pad"], geo["Wc"], geo["Q"], geo["buckets"]

    nc = bacc.Bacc("TRN2", target_bir_lowering=False, debug=False,
                   num_devices=NCORES)
    xin = nc.dram_tensor("xin", [128, NWIN, PH, Wc * 2], _dt.uint8,
                         kind="ExternalInput").ap()
    xout = nc.dram_tensor("out", [128, Q * 2], _dt.uint8,
                          kind="ExternalOutput").ap()

    # last window whose scan a bucket's extraction lattice depends on
    def last_win(B, q, l):
        return min(NWIN - 1, (B + q * l - 1) // Wc)

    subs_after = {w: [] for w in range(NWIN)}
    for bkt in buckets:
        l, q, B, O = bkt
        subs_after[last_win(B, q, l)].append(bkt)

    with tile.TileContext(nc) as tc:
        with tc.tile_pool(name="pp", bufs=1) as pool, \
             tc.tile_pool(name="xp", bufs=3) as xpool, \
             tc.tile_pool(name="ap", bufs=2) as apool:
            S = pool.tile([128, C4pad], _dt.float32, tag="S")
            ot = pool.tile([128, Q], _dt.float16, tag="o")
            for w in range(NWIN):
                a, b = w * Wc, (w + 1) * Wc
                xt = xpool.tile([128, PH * Wc], _dt.bfloat16, tag="x",
                                name="xt")
                nc.sync.dma_start(
                    out=xt[:, 0:2 * Wc],
                    in_=xin[:, w, 0:2, :].bitcast(_dt.bfloat16))
                nc.scalar.dma_start(
                    out=xt[:, 2 * Wc:4 * Wc],
                    in_=xin[:, w, 2:4, :].bitcast(_dt.bfloat16))
                a01 = apool.tile([128, Wc], _dt.bfloat16, tag="a01")
                a23 = apool.tile([128, Wc], _dt.bfloat16, tag="a23")
                nc.vector.tensor_add(out=a01[:], in0=xt[:, 0:Wc],
                                     in1=xt[:, Wc:2 * Wc])
                nc.vector.tensor_add(out=a23[:], in0=xt[:, 2 * Wc:3 * Wc],
                                     in1=xt[:, 3 * Wc:4 * Wc])
                init = 0.0 if w == 0 else S[:, a - 1:a]
                nc.vector._custom_dve(_PAIR_ADD_SCAN, out=S[:, a:b],
                                      in0=a01[:], in1=a23[:], s0=init)
                # strided lattice subtracts run on the otherwise-idle GPSIMD
                # as soon as the last covering scan is done
                for (l, q, B, O) in subs_after[w]:
                    e0 = B + l - 1
                    nc.gpsimd.tensor_sub(
                        out=ot[:, O:O + q],
                        in0=S[:, e0:e0 + q * l:l],
                        in1=S[:, B - 1:B - 1 + q * l:l])
            nc.scalar.dma_start(out=xout[:], in_=ot[:].bitcast(_dt.uint8))
    nc.compile()
    _CACHE["nc"] = nc
    return nc


def postprocess(results, meta):
    """Pull per-run sums from the compacted device outputs, expand to rows."""
    table = np.zeros((NUM_SEGMENTS, N_ARY), dtype=np.float32)
    core = meta["core_of_run"]
    group = meta["group_of_run"]
    outcol = meta["outcol_of_run"]
    for i in range(NCORES):
        O = results[i]["out"].view(F16).astype(np.float32)       # [128, Q]
        O = O.reshape(GROUPS, 32, meta["Q"])
        m = core == i
        table[meta["seg_of_run"][m]] = O[group[m], :, outcol[m]]
    return table[meta["seg"]]


def run(in_maps, trace=False, trace_kwargs=None):
    _ensure_axon_hooks()
    from concourse.bass_utils import run_bass_kernel_spmd
    nc = build_bass()
    return run_bass_kernel_spmd(nc, in_maps, core_ids=list(range(NCORES)),
                                trace=trace, **(trace_kwargs or {}))


def kernel(inputs, idx_inputs, cat_mask, numeric_mask):
    in_maps, meta = prepare(inputs, idx_inputs, cat_mask, numeric_mask)
    res = run(in_maps, trace=False)
    return postprocess(res.results, meta)
